# revision 26
# baseline (speedup 1.0000x reference)
"""Trainium2 Bass kernel for nn_CorrOptDiMP: DiMP correlation-filter
steepest-descent optimizer (num_iter iterations), data-parallel over the
16 sequences across 8 NeuronCores (2 sequences per core).

End-to-end wall-clock over the axon tunnel is dominated by wire bytes +
per-call dispatch, so this version minimizes both:
  - ONE packed fp16 input blob per core (~1.0 MB): feat, filt (natural
    layouts, no host transposes) and the three 43x43 base distance maps.
  - All derived tensors built on device: f2^T and w0^T via PE-transposes,
    the [484,484] unfolded maps via negative-stride gather DMAs through a
    DRAM scratch (the full map is point-symmetric, so the [f,x]-major
    gather can be read back x-major for free), c1/c2/sw^2 via ACT/DVE.
  - Output is the fp16 weight DELTA in natural [f,c] layout (~0.5 MB per
    core); host adds it to the original fp32 filt.
  - The jitted shard_map executable is cached per num_iter (the stock
    runner re-jits every call); output buffers are created on-device by a
    tiny jitted zeros fn, so no zero-buffer upload either.

Per-iteration math (per sequence), all on device:
    scoresT[x,f] = sum_c f2[c,x] * wT[c,f]            (PE, fp16 in)
    m = c1*sign(s) + c2      (c1=0.5(1-a), c2=0.5(1+a))
    res = m * (sw2 * (m*s - label))                   (DVE/GPSIMD)
    wgT[c,f] = sum_x f2T[x,c]*res[x,f] + reg*wT       (PE + DVE reg-add)
    num[f] = sum_c wgT^2 ; den[f] = sum_x (sw*m*sgT)^2 + reg*num
    alpha = num / max(den,1e-8)                       (DVE reciprocal)
    wT -= step * alpha * wgT                          (fp32 master)
"""

import sys
from concurrent.futures import ThreadPoolExecutor
from contextlib import ExitStack

import numpy as np

for _p in ("/opt/trn_rl_repo",):
    if _p not in sys.path:
        sys.path.insert(0, _p)

import bass_rust  # noqa: E402
import concourse.bass as bass  # noqa: E402
import concourse.tile as tile  # noqa: E402
from concourse import bacc, mybir  # noqa: E402

NUM_BINS = 10
BIN_DISP = 0.5
MIN_REG = 1e-5
H = W = 22
SZ = 2 * H - 1     # 43
S = 16
C = 256
F = H * W          # 484 filters
X = H * W          # 484 spatial locations
NCORES = 8
SPC = S // NCORES  # sequences per core = 2
XT = 121           # x-tile (partition) size; 484 = 4 * 121
NXT = 4

SEQB = 2 * 128 * F          # 123904 elems per sequence slab (f2 or w0)
F2_OFF = 0
W0_OFF = SPC * SEQB         # 247808
FM_OFF = 2 * SPC * SEQB     # 495616
FM_LEN = 3 * SZ * SZ        # 5547
L_BIG = FM_OFF + FM_LEN + 1  # 501164 (pad to even)
SM_LEN = 505                # step[128] + onesc[128*2] + onesx[121]

dt16 = mybir.dt.float16
dt32 = mybir.dt.float32
dtr = mybir.dt.float32r
dti8 = mybir.dt.int8
AF = mybir.ActivationFunctionType
OP = mybir.AluOpType

_RUN_CACHE: dict = {}


def _xsl(xt):
    return slice(XT * xt, XT * (xt + 1))


def _cap(handle, offset, dims):
    """Custom access pattern on a DRAM tensor handle: dims = [[stride, size], ...]."""
    return bass_rust.AP(tensor=handle, offset=offset, ap=[list(d) for d in dims])


def _build_fm(label_w, mask_w, spatial_w):
    """Host: the three [43,43] base maps (float64 precision)."""
    cy = SZ // 2
    k0 = np.arange(SZ, dtype=np.float64)[:, None]
    k1 = np.arange(SZ, dtype=np.float64)[None, :]
    dist = np.sqrt((k0 - cy) ** 2 + (k1 - cy) ** 2)
    bins = np.arange(NUM_BINS, dtype=np.float64)[:, None, None]
    bd = dist[None] / BIN_DISP - bins
    lower = np.maximum(1.0 - np.abs(bd[:-1]), 0.0)
    last = np.clip(1.0 + bd[-1:], 0.0, 1.0)
    dmap = np.concatenate([lower, last], axis=0)  # [10, 43, 43]

    label = np.einsum("bhw,b->hw", dmap, label_w.astype(np.float64))
    amap = 1.0 / (1.0 + np.exp(-np.einsum("bhw,b->hw", dmap, mask_w.astype(np.float64))))
    sw = np.einsum("bhw,b->hw", dmap, spatial_w.astype(np.float64))
    return np.stack([label, amap, sw]).astype(np.float16)  # [3, 43, 43]


def _iteration(nc, pools, cv, s, w_cur):
    """Emit one optimizer iteration for sequence s. Returns new wT tile."""
    work, wpool, sm, pss, psw = pools

    # fp16 copy of master weights for the scores matmul
    w16 = work.tile([128, 2, 484], dt16, tag="w16", name=f"w16_{s}")
    nc.scalar.activation(w16[:, :, :], w_cur[:, :, :], AF.Copy)

    sgn = work.tile([121, NXT, 484], dt16, tag="sgn", name=f"sgn_{s}")
    s16 = work.tile([121, NXT, 484], dt16, tag="s16", name=f"s16_{s}")
    for k in range(2):  # two 2-bank psum chunks over the 4 x-tiles
        ps = pss.tile([121, 2, 512], dt32, tag="pss", name=f"ps_s{s}_{k}")
        for j in range(2):
            xt = 2 * k + j
            for ct in range(2):
                nc.tensor.matmul(
                    ps[:, j, 0:484],
                    lhsT=cv["f2"][:, s, ct, _xsl(xt)],
                    rhs=w16[:, ct, :],
                    start=(ct == 0),
                    stop=(ct == 1),
                )
        pv = ps[:, :, 0:484]
        nc.scalar.activation(sgn[:, 2 * k : 2 * k + 2, :], pv, AF.Sign)
        nc.scalar.activation(s16[:, 2 * k : 2 * k + 2, :], pv, AF.Copy)

    # m = c1*sgn + c2 ; res = m * (sw2 * (m*s - label))
    t0 = work.tile([121, NXT, 484], dt16, tag="t0", name=f"t0_{s}")
    nc.vector.tensor_tensor(t0, cv["c1"], sgn, OP.mult)
    m = work.tile([121, NXT, 484], dt16, tag="m", name=f"m_{s}")
    nc.vector.tensor_tensor(m, t0, cv["c2"], OP.add)
    ms = work.tile([121, NXT, 484], dt16, tag="ms", name=f"ms_{s}")
    nc.vector.tensor_tensor(ms, m, s16, OP.mult)
    qq = work.tile([121, NXT, 484], dt16, tag="qq", name=f"qq_{s}")
    nc.gpsimd.tensor_tensor(qq, ms, cv["lbl"], OP.subtract)
    uu = work.tile([121, NXT, 484], dt16, tag="uu", name=f"uu_{s}")
    nc.gpsimd.tensor_tensor(uu, cv["sw2"], qq, OP.mult)
    res = work.tile([121, NXT, 484], dt16, tag="res", name=f"res_{s}")
    nc.vector.tensor_tensor(res, m, uu, OP.mult)

    # wgT = f2T @ res + reg * wT   (reg-term via on-device reg*I matmul)
    pw = psw.tile([128, 2, 512], dt32, tag="psw", name=f"ps_w{s}")
    for ct in range(2):
        for xt in range(NXT):
            nc.tensor.matmul(
                pw[:, ct, 0:484],
                lhsT=cv["f2t"][:, s, xt, 128 * ct : 128 * (ct + 1)],
                rhs=res[:, xt, :],
                start=(xt == 0),
                stop=False,
            )
        nc.tensor.matmul(
            pw[:, ct, 0:484],
            lhsT=cv["regeye"],
            rhs=w_cur[:, ct, :],
            start=False,
            stop=True,
        )
    pwv = pw[:, :, 0:484]
    wg16 = work.tile([128, 2, 484], dt16, tag="wg16", name=f"wg16_{s}")
    nc.scalar.activation(wg16, pwv, AF.Copy)
    sqw = work.tile([128, 2, 484], dtr, tag="sqw", name=f"sqw_{s}")
    nc.scalar.activation(sqw, pwv, AF.Square)

    # sgT = f2 @ wg16 ; sgs = sw * m * sg ; sqg = sgs^2
    sg16 = work.tile([121, NXT, 484], dt16, tag="sg16", name=f"sg16_{s}")
    for k in range(2):
        ps = pss.tile([121, 2, 512], dt32, tag="pss", name=f"ps_g{s}_{k}")
        for j in range(2):
            xt = 2 * k + j
            for ct in range(2):
                nc.tensor.matmul(
                    ps[:, j, 0:484],
                    lhsT=cv["f2"][:, s, ct, _xsl(xt)],
                    rhs=wg16[:, ct, :],
                    start=(ct == 0),
                    stop=(ct == 1),
                )
        nc.scalar.activation(sg16[:, 2 * k : 2 * k + 2, :], ps[:, :, 0:484], AF.Copy)
    sgm = work.tile([121, NXT, 484], dt16, tag="sgm", name=f"sgm_{s}")
    nc.vector.tensor_tensor(sgm, m, sg16, OP.mult)
    sgs = work.tile([121, NXT, 484], dt16, tag="sgs", name=f"sgs_{s}")
    nc.gpsimd.tensor_tensor(sgs, cv["sw"], sgm, OP.mult)
    sqg = work.tile([121, NXT, 484], dtr, tag="sqg", name=f"sqg_{s}")
    nc.vector.tensor_tensor(sqg, sgs, sgs, OP.mult)

    # num[f] = sum_c wg^2; den[f] = sum_x sgs^2 + reg*num  (ones-reduce on PE)
    pnd = psw.tile([1, 2, 512], dt32, tag="psw", name=f"ps_nd{s}")
    for ct in range(2):
        nc.tensor.matmul(
            pnd[0:1, 0, 0:484],
            lhsT=cv["onesc"][:, 0:1],
            rhs=sqw[:, ct, :],
            start=(ct == 0),
            stop=(ct == 1),
        )
    for ct in range(2):
        nc.tensor.matmul(
            pnd[0:1, 1, 0:484],
            lhsT=cv["onesc"][:, 1:2],
            rhs=sqw[:, ct, :],
            start=(ct == 0),
            stop=False,
        )
    for xt in range(NXT):
        nc.tensor.matmul(
            pnd[0:1, 1, 0:484],
            lhsT=cv["onesx"],
            rhs=sqg[:, xt, :],
            start=False,
            stop=(xt == NXT - 1),
        )

    # alpha = num / max(den, 1e-8) via DVE reciprocal
    dn = sm.tile([1, 484], dt32, tag="dn", name=f"dn_{s}")
    nc.vector.tensor_scalar(dn, pnd[0:1, 1, 0:484], 1e-8, None, OP.max)
    rcp = sm.tile([1, 484], dt32, tag="rcp", name=f"rcp_{s}")
    nc.vector.reciprocal(rcp, dn)
    alpha = sm.tile([1, 484], dtr, tag="alpha", name=f"alpha_{s}")
    nc.vector.tensor_tensor(alpha, pnd[0:1, 0, 0:484], rcp, OP.mult)

    # broadcast step*alpha over partitions via 1-row matmul, then update
    pb = psw.tile([128, 2, 512], dt32, tag="psw", name=f"ps_b{s}")
    nc.tensor.matmul(
        pb[:, 0, 0:484],
        lhsT=cv["stepo"],
        rhs=alpha,
        start=True,
        stop=True,
    )
    w_new = wpool.tile([128, 2, 484], dt32, tag="w32", name=f"w_{s}")
    for ct in range(2):
        t = work.tile([128, 484], dt32, tag="upd", name=f"upd_{s}_{ct}")
        nc.vector.scalar_tensor_tensor(
            t, pb[:, 0, 0:484], 1.0, wg16[:, ct, :], OP.mult, OP.mult
        )
        nc.vector.tensor_tensor(w_new[:, ct, :], w_cur[:, ct, :], t, OP.subtract)
    return w_new


def _build_nc(num_iter):
    nc = bacc.Bacc("TRN2", target_bir_lowering=False, debug=False)

    d_big = nc.dram_tensor("big", [L_BIG], dt16, kind="ExternalInput")
    d_small = nc.dram_tensor("small", [SM_LEN], dtr, kind="ExternalInput")
    d_out = nc.dram_tensor("dout", [SPC, F, C], dti8, kind="ExternalOutput")
    d_osc = nc.dram_tensor("dosc", [SPC, F], dt32, kind="ExternalOutput")
    d_scr = nc.dram_tensor("mscr", [3, F, X], dt16, kind="Internal")
    d_ident = nc.inline_tensor(np.eye(128, dtype=np.float16), name="ident")
    bigt = d_big[:].tensor

    with tile.TileContext(nc) as tc, ExitStack() as ctx:
        consts = ctx.enter_context(tc.tile_pool(name="consts", bufs=1))
        w0pool = ctx.enter_context(tc.tile_pool(name="w0pool", bufs=SPC))
        wpool = ctx.enter_context(tc.tile_pool(name="wpool", bufs=4))
        work = ctx.enter_context(tc.tile_pool(name="work", bufs=2))
        sm = ctx.enter_context(tc.tile_pool(name="sm", bufs=2))
        epi = ctx.enter_context(tc.tile_pool(name="epi", bufs=1))
        pss = ctx.enter_context(tc.tile_pool(name="pss", bufs=2, space="PSUM"))
        psw = ctx.enter_context(tc.tile_pool(name="psw", bufs=2, space="PSUM"))

        cv = {}
        ident = consts.tile([128, 128], dt16, name="ident_sb")
        nc.sync.dma_start(out=ident, in_=d_ident[:])

        # f2 [c-part, x] and w0 natural [f-part, c], packed in the blob
        f2 = consts.tile([128, SPC, 2, 484], dt16, name="f2_sb")
        for s in range(SPC):
            nc.sync.dma_start(
                out=f2[:, s],
                in_=_cap(bigt, F2_OFF + s * SEQB, [[484, 128], [61952, 2], [1, 484]]),
            )
        cv["f2"] = f2
        w0n = consts.tile([121, SPC, 4, 256], dt16, name="w0n_sb")
        for s in range(SPC):
            nc.sync.dma_start(
                out=w0n[:, s],
                in_=_cap(bigt, W0_OFF + s * SEQB, [[256, 121], [30976, 4], [1, 256]]),
            )

        # unfolded maps: per-row gather into DRAM scratch, then x-major load
        # (the full map is point-symmetric: scratch rows are f, read as x)
        scrt = d_scr[:].tensor
        for i in range(3):
            for li in range(H):
                # dims [ki, lj, kj]: scr[(li,lj), (ki,kj)] = fm[21-li+ki, 21-lj+kj]
                nc.sync.dma_start(
                    out=_cap(
                        scrt,
                        i * F * X + li * H * X,
                        [[H, H], [X, H], [1, H]],
                    ),
                    in_=_cap(
                        bigt,
                        FM_OFF + i * SZ * SZ + (H - 1 - li) * SZ + (W - 1),
                        [[SZ, H], [-1, H], [1, H]],
                    ),
                )
        mt = []
        for i, nm in enumerate(("lbl", "amap", "swm")):
            t = consts.tile([121, NXT, 484], dt16, name=f"{nm}_sb")
            nc.sync.dma_start(out=t, in_=d_scr[i].rearrange("(t p) f -> p t f", t=NXT))
            mt.append(t)
        cv["lbl"], amap, cv["sw"] = mt
        c1 = consts.tile([121, NXT, 484], dt16, name="c1_sb")
        nc.scalar.activation(c1, amap, AF.Copy, scale=-0.5, bias=0.5)
        cv["c1"] = c1
        c2 = consts.tile([121, NXT, 484], dt16, name="c2_sb")
        nc.scalar.activation(c2, amap, AF.Copy, scale=0.5, bias=0.5)
        cv["c2"] = c2
        sw2 = consts.tile([121, NXT, 484], dt16, name="sw2_sb")
        nc.vector.tensor_tensor(sw2, cv["sw"], cv["sw"], OP.mult)
        cv["sw2"] = sw2

        # small consts: step row, [ones | reg] cols, x-ones
        stepo = consts.tile([1, 128], dtr, name="stepo_sb")
        nc.sync.dma_start(out=stepo, in_=d_small[0:128].unsqueeze(0))
        cv["stepo"] = stepo
        onesc = consts.tile([128, 2], dtr, name="onesc_sb")
        nc.sync.dma_start(out=onesc, in_=d_small[128:384].rearrange("(p t) -> p t", p=128))
        cv["onesc"] = onesc
        onesx = consts.tile([121, 1], dtr, name="onesx_sb")
        nc.sync.dma_start(out=onesx, in_=d_small[384:505].unsqueeze(1))
        cv["onesx"] = onesx
        regsc = consts.tile([128, 1], dt32, name="regsc_sb")
        nc.vector.tensor_copy(regsc, onesc[:, 1:2])
        regeye = consts.tile([128, 128], dt32, name="regeye_sb")
        nc.scalar.activation(regeye, ident, AF.Copy, scale=regsc)
        cv["regeye"] = regeye

        # PE-transpose w0 -> fp32 master [c-part, f]; f2 -> f2T [x-part, c]
        f2t = consts.tile([121, SPC, 4, 256], dt16, name="f2t_sb")
        cv["f2t"] = f2t
        w0_32 = {}
        for s in range(SPC):
            pw = psw.tile([128, 2, 512], dt32, tag="psw", name=f"pt_w{s}")
            for ct in range(2):
                for xt in range(NXT):
                    nc.tensor.matmul(
                        pw[:, ct, 121 * xt : 121 * (xt + 1)],
                        lhsT=w0n[:, s, xt, 128 * ct : 128 * (ct + 1)],
                        rhs=ident[0:121, 0:121],
                        start=True,
                        stop=True,
                    )
            t = w0pool.tile([128, 2, 484], dt32, tag="w0", name=f"w0_{s}")
            nc.scalar.activation(t, pw[:, :, 0:484], AF.Copy)
            w0_32[s] = t

            pf = pss.tile([121, 2, 512], dt32, tag="pss", name=f"pt_f{s}")
            for xt in range(NXT):
                for ct in range(2):
                    cbase = 256 * (xt % 2) + 128 * ct
                    nc.tensor.matmul(
                        pf[:, xt // 2, cbase : cbase + 128],
                        lhsT=f2[:, s, ct, _xsl(xt)],
                        rhs=ident,
                        start=True,
                        stop=True,
                    )
            nc.scalar.activation(
                f2t[:, s].rearrange("p t c -> p (t c)"),
                pf.rearrange("p a b -> p (a b)"),
                AF.Copy,
            )

        pools = (work, wpool, sm, pss, psw)
        w_cur = dict(w0_32)
        for it in range(num_iter):
            for s in range(SPC):
                w_cur[s] = _iteration(nc, pools, cv, s, w_cur[s])

        # delta = w_final - w0, PE-transpose back to natural [f, c] fp16
        for s in range(SPC):
            d16 = epi.tile([128, 2, 484], dt16, tag=f"d16_{s}", name=f"d16_{s}")
            nc.vector.tensor_tensor(d16, w_cur[s], w0_32[s], OP.subtract)
            pd = pss.tile([121, 2, 512], dt32, tag="pss", name=f"pt_d{s}")
            for xt in range(NXT):
                for ct in range(2):
                    cbase = 256 * (xt % 2) + 128 * ct
                    nc.tensor.matmul(
                        pd[:, xt // 2, cbase : cbase + 128],
                        lhsT=d16[:, ct, _xsl(xt)],
                        rhs=ident,
                        start=True,
                        stop=True,
                    )
            dsb = epi.tile([121, 4, 256], dt16, tag=f"dsb_{s}", name=f"dsb_{s}")
            nc.scalar.activation(
                dsb.rearrange("p t c -> p (t c)"),
                pd.rearrange("p a b -> p (a b)"),
                AF.Copy,
            )
            # int8 quantization with per-(f-row over c) scales
            mx = epi.tile([121, 4], dt32, tag=f"mx_{s}", name=f"mx_{s}")
            nc.vector.tensor_reduce(
                mx, dsb, mybir.AxisListType.X, OP.max, apply_absolute_value=True
            )
            sc = epi.tile([121, 4], dt32, tag=f"sc_{s}", name=f"sc_{s}")
            nc.vector.tensor_scalar(sc, mx, 1.0 / 126.0, 1e-30, OP.mult, OP.max)
            rsc = epi.tile([121, 4], dt32, tag=f"rsc_{s}", name=f"rsc_{s}")
            nc.vector.reciprocal(rsc, sc)
            q = epi.tile([121, 4, 256], dti8, tag=f"q_{s}", name=f"q_{s}")
            for t in range(NXT):
                nc.vector.tensor_scalar(
                    q[:, t, :], dsb[:, t, :], rsc[:, t : t + 1], None, OP.mult
                )
            nc.sync.dma_start(
                out=d_out[s].rearrange("(t p) c -> p t c", t=NXT), in_=q
            )
            nc.sync.dma_start(
                out=d_osc[s].rearrange("(t p) -> p t", t=NXT), in_=sc
            )

    nc.compile()
    return nc


def _get_runner(n_it):
    if n_it in _RUN_CACHE:
        return _RUN_CACHE[n_it]

    import jax
    import jax.numpy as jnp
    from jax.sharding import Mesh, NamedSharding, PartitionSpec

    try:
        from jax.experimental.shard_map import shard_map
    except ImportError:  # newer jax
        from jax import shard_map
    from concourse import bass2jax

    bass2jax.install_neuronx_cc_hook()
    nc = _build_nc(n_it)

    partition_name = nc.partition_id_tensor.name if nc.partition_id_tensor else None
    in_names, out_names, out_avals = [], [], []
    for alloc in nc.m.functions[0].allocations:
        if not isinstance(alloc, mybir.MemoryLocationSet):
            continue
        name = alloc.memorylocations[0].name
        if alloc.kind == "ExternalInput":
            if name != partition_name:
                in_names.append(name)
        elif alloc.kind == "ExternalOutput":
            assert alloc.tensor_shape is not None and alloc.dtype is not None
            out_names.append(name)
            out_avals.append(
                jax.core.ShapedArray(tuple(alloc.tensor_shape), mybir.dt.np(alloc.dtype))
            )
    n_params = len(in_names)
    n_outs = len(out_names)
    in_names_full = list(in_names) + list(out_names)
    if partition_name is not None:
        in_names_full.append(partition_name)
    donate = tuple(range(n_params, n_params + n_outs))

    def _body(*args):
        operands = list(args)
        if partition_name is not None:
            operands.append(bass2jax.partition_id_tensor())
        outs = bass2jax._bass_exec_p.bind(
            *operands,
            out_avals=tuple(out_avals),
            in_names=tuple(in_names_full),
            out_names=tuple(out_names),
            lowering_input_output_aliases=(),
            sim_require_finite=True,
            sim_require_nnan=True,
            nc=nc,
        )
        return tuple(outs)

    devices = jax.devices()[:NCORES]
    assert len(devices) == NCORES
    mesh = Mesh(np.asarray(devices), ("core",))
    sharding = NamedSharding(mesh, PartitionSpec("core"))
    in_specs = (PartitionSpec("core"),) * (n_params + n_outs)
    out_specs = (PartitionSpec("core"),) * n_outs
    sharded = jax.jit(
        shard_map(
            _body, mesh=mesh, in_specs=in_specs, out_specs=out_specs, check_rep=False
        ),
        donate_argnums=donate,
        keep_unused=True,
    )
    zeros_fn = jax.jit(
        lambda: tuple(
            jnp.zeros((NCORES * av.shape[0],) + tuple(av.shape[1:]), av.dtype)
            for av in out_avals
        ),
        out_shardings=tuple(sharding for _ in out_avals),
    )
    bundle = (nc, in_names, out_names, sharded, zeros_fn)
    _RUN_CACHE[n_it] = bundle
    return bundle


_HOST_BUFS: dict = {}
_POOL = ThreadPoolExecutor(8)


def _host_pack(filt, feat, log_step_length, filter_reg, label_w, mask_w, spatial_w):
    step = float(np.exp(np.float32(log_step_length.reshape(-1)[0])))
    fr = float(np.float32(filter_reg.reshape(-1)[0]))
    reg = max(fr * fr, MIN_REG**2)

    if "big" not in _HOST_BUFS:
        _HOST_BUFS["big"] = np.empty((NCORES, L_BIG), np.float16)
    big = _HOST_BUFS["big"]
    feat2 = feat.reshape(S, C * X)
    filt2 = filt.reshape(S, F * C)

    def pack_core(core):
        sl = slice(core * SPC, (core + 1) * SPC)
        big[core, F2_OFF : F2_OFF + SPC * SEQB].reshape(SPC, SEQB)[:] = feat2[sl]
        big[core, W0_OFF : W0_OFF + SPC * SEQB].reshape(SPC, SEQB)[:] = filt2[sl]

    futs = [_POOL.submit(pack_core, c) for c in range(NCORES)]
    fm = _build_fm(label_w, mask_w, spatial_w).reshape(-1)  # fp16 [5547]
    big[:, FM_OFF : FM_OFF + FM_LEN] = fm[None, :]
    big[:, -1] = 0

    small = np.empty((NCORES, SM_LEN), np.float32)
    small[:, 0:128] = step
    onesc = np.stack(
        [np.ones(128, np.float32), np.full(128, reg, np.float32)], axis=1
    ).reshape(-1)
    small[:, 128:384] = onesc[None, :]
    small[:, 384:505] = 1.0
    for f in futs:
        f.result()
    return big, small


def kernel(filt, feat, log_step_length, filter_reg, label_w, mask_w, spatial_w,
           num_iter, _trace=False, _trace_kwargs=None):
    filt = np.asarray(filt, np.float32)
    feat = np.asarray(feat, np.float32)
    log_step_length = np.asarray(log_step_length, np.float32)
    filter_reg = np.asarray(filter_reg, np.float32)
    label_w = np.asarray(label_w, np.float32)
    mask_w = np.asarray(mask_w, np.float32)
    spatial_w = np.asarray(spatial_w, np.float32)
    n_it = int(np.asarray(num_iter).reshape(-1)[0]) if np.asarray(num_iter).size else int(num_iter)

    if n_it <= 0:
        return filt.copy()

    nc, in_names, out_names, sharded, zeros_fn = _get_runner(n_it)
    big, small = _host_pack(
        filt, feat, log_step_length, filter_reg, label_w, mask_w, spatial_w
    )
    arrs = {"big": big.reshape(-1), "small": small.reshape(-1)}
    ins = [arrs[nm] for nm in in_names]
    z = zeros_fn()
    outs = sharded(*ins, *z)
    for o in outs:
        try:
            o.copy_to_host_async()
        except Exception:
            pass
    oi = {nm: i for i, nm in enumerate(out_names)}
    q = np.asarray(outs[oi["dout"]])   # [S, F, C] int8
    sc = np.asarray(outs[oi["dosc"]])  # [S, F] f32

    ret = np.empty((S, F, C, 1, 1), np.float32)
    r3 = ret.reshape(S, F, C)
    filt3 = filt.reshape(S, F, C)

    def add_seq(s):
        np.multiply(q[s], sc[s][:, None], out=r3[s], casting="unsafe")
        np.add(r3[s], filt3[s], out=r3[s])

    list(_POOL.map(add_seq, range(S)))
    if _trace:
        return ret, None
    return ret
